# revision 1
# baseline (speedup 1.0000x reference)
"""Trainium2 Bass kernel for BiAttention (b=8, n=m=1024, d=512).

Sharding: data-parallel over batch — one batch element per NeuronCore,
8 cores, no cross-core communication.

Per-core algorithm (softmax shift-invariance lets the Linear(3d,1) row/col
terms, the bias, and both padding masks fold into per-partition exponent
biases, so no max-subtraction pass and no partition-axis reductions are
needed; logits are ~N(0,1) so raw exp is safe):

  sim      = (x1*w3) @ x2^T                     (n, m)   [tri term only]
  s1[n]    = x1 @ w1,   s2[m] = x2 @ w2
  b1[n]    = s1 + (-30000 if x1_mask else 0)
  b2[m]    = s2 + (-30000 if x2_mask else 0)
  E_g      = exp(sim   + b1[:,None])            (n, m)   row-weighted, n-masked
  ET_g     = exp(sim^T + b2[:,None])            (m, n)   col-weighted, m-masked
  U_row    = ET_g^T @ x2          (n, d);  den1[n] = sum_m ET_g[m,n]
  U_col    = E_g^T  @ x1          (m, d);  den2[m] = sum_n E_g[n,m]
  c2q      = U_row / den1
  q2c      = U_col / den2
  V        = ET_g^T @ q2c         (n, d)
  q2c_att  = V / den1
  out      = [x1, c2q, x1*c2q, x1*q2c_att]      (n, 4d)

All big matmuls run in float32r (full-rate PE). s1 is recovered from the
w3-scaled transposed copy via u1 = w1/w3.
"""

import numpy as np
from contextlib import ExitStack

import concourse.bacc as bacc
import concourse.tile as tile
import concourse.mybir as mybir
from concourse.bass_utils import run_bass_kernel_spmd
from concourse.masks import make_identity

F32 = mybir.dt.float32
U8 = mybir.dt.uint8
R = mybir.dt.float32r
BF16 = mybir.dt.bfloat16
EXP = mybir.ActivationFunctionType.Exp

P = 128
N = 1024          # x1 rows
M = 1024          # x2 rows
D = 512           # feature dim
NT, MT, DC = N // P, M // P, D // P
NEGB = -30000.0   # exp(x + NEGB) == 0.0 exactly for |x| < 80

N_CORES = 8

_CACHE = {}


def _build(mm_dtype=R):
    nc = bacc.Bacc("TRN2", target_bir_lowering=False, debug=False)
    x1d = nc.dram_tensor("x1", [N, D], F32, kind="ExternalInput").ap()
    x2d = nc.dram_tensor("x2", [M, D], F32, kind="ExternalInput").ap()
    m1d = nc.dram_tensor("x1_mask", [N], U8, kind="ExternalInput").ap()
    m2d = nc.dram_tensor("x2_mask", [M], U8, kind="ExternalInput").ap()
    wd = nc.dram_tensor("W", [3 * D], F32, kind="ExternalInput").ap()
    outd = nc.dram_tensor("out", [N, 4 * D], F32, kind="ExternalOutput").ap()

    x1r_d = x1d.rearrange("(t p) d -> p t d", p=P)
    x2r_d = x2d.rearrange("(t p) d -> p t d", p=P)
    out_r = outd.rearrange("(t p) e -> p t e", p=P)

    with tile.TileContext(nc) as tc, ExitStack() as ctx:
        const = ctx.enter_context(tc.tile_pool(name="const", bufs=1))
        big = ctx.enter_context(tc.tile_pool(name="big", bufs=1))
        rows = ctx.enter_context(tc.tile_pool(name="rows", bufs=1))
        work = ctx.enter_context(tc.tile_pool(name="work", bufs=3))
        psb = ctx.enter_context(tc.tile_pool(name="psb", bufs=2, space="PSUM"))
        pss = ctx.enter_context(tc.tile_pool(name="pss", bufs=4, space="PSUM"))
        dram = ctx.enter_context(tc.tile_pool(name="dram", bufs=1, space="DRAM"))

        # ---------- constants ----------
        ident = const.tile([P, P], F32)
        make_identity(nc, ident)
        wcols = const.tile([P, 12], F32)  # W[(c p)] -> (p, c): w1=0:4 w2=4:8 w3=8:12
        nc.sync.dma_start(wcols[:], wd.rearrange("(c p) -> p c", p=P))
        w3rec = const.tile([P, 4], F32)
        nc.vector.reciprocal(w3rec[:], wcols[:, 8:12])
        u1r = const.tile([P, 4], mm_dtype)  # w1/w3 — recovers s1 from w3-scaled x1T
        nc.vector.tensor_mul(u1r[:], wcols[:, 0:4], w3rec[:])
        w2r = const.tile([P, 4], mm_dtype)
        nc.vector.tensor_copy(w2r[:], wcols[:, 4:8])
        ones_f = const.tile([P, 1], F32)
        nc.vector.memset(ones_f[:], 1.0)
        ones_r = const.tile([P, 1], mm_dtype)
        nc.vector.tensor_copy(ones_r[:], ones_f[:])

        # masks -> (1, N) exponent offsets (0 valid / NEGB padded)
        m1row = rows.tile([1, N], U8)
        nc.sync.dma_start(m1row[:], m1d.rearrange("(a n) -> a n", a=1))
        m2row = rows.tile([1, M], U8)
        nc.sync.dma_start(m2row[:], m2d.rearrange("(a n) -> a n", a=1))
        logm1 = rows.tile([1, N], F32)
        nc.vector.tensor_scalar_mul(logm1[:], m1row[:], NEGB)
        logm2 = rows.tile([1, M], F32)
        nc.vector.tensor_scalar_mul(logm2[:], m2row[:], NEGB)

        # ---------- natural x1 (resident) ----------
        x1n = big.tile([P, NT, D], F32)
        nc.sync.dma_start(x1n[:], x1r_d)
        # out block 0 = x1, straight dram->dram
        nc.sync.dma_start(out_r[:, :, 0:D], x1r_d)

        x1w3T = big.tile([P, DC, N], mm_dtype)  # (d_chunk, n) of x1*w3, transposed
        x2T = big.tile([P, DC, M], mm_dtype)    # (d_chunk, m) of x2, transposed
        x1aug = big.tile([P, NT, D], mm_dtype)  # f32r cast of x1 (U_col rhs)
        x2aug = big.tile([P, MT, D], mm_dtype)  # f32r cast of x2 (U_row rhs)

        # ---------- transposes (PE) + casts ----------
        for t in range(NT):
            for c in range(DC):
                ptr = pss.tile([P, P], F32, tag="ps_sm")
                nc.tensor.transpose(ptr[:], x1n[:, t, c * P:(c + 1) * P], ident[:])
                # evict fused with w3 scaling (w3 is per-partition in (d, n) layout)
                nc.vector.tensor_scalar_mul(
                    x1w3T[:, c, t * P:(t + 1) * P], ptr[:], wcols[:, 8 + c:9 + c])
            nc.vector.tensor_copy(x1aug[:, t, :], x1n[:, t, :])
        for t in range(MT):
            x2t = work.tile([P, D], F32, tag="x2s")
            nc.sync.dma_start(x2t[:], x2r_d[:, t, :])
            for c in range(DC):
                ptr = pss.tile([P, P], F32, tag="ps_sm")
                nc.tensor.transpose(ptr[:], x2t[:, c * P:(c + 1) * P], ident[:])
                nc.vector.tensor_copy(x2T[:, c, t * P:(t + 1) * P], ptr[:])
            nc.vector.tensor_copy(x2aug[:, t, :], x2t[:])

        # ---------- s1, s2 rows -> bias columns (dram round-trip transpose) ----------
        b1col = const.tile([P, NT], F32)
        b2col = const.tile([P, MT], F32)
        for (name, lhs, rhsT, logm, bcol, nt) in (
            ("b1", u1r, x1w3T, logm1, b1col, NT),
            ("b2", w2r, x2T, logm2, b2col, MT),
        ):
            brow = rows.tile([1, P * nt], F32, tag=f"{name}row")
            for h in range(nt * P // 512):
                ps_s = pss.tile([1, 512], F32, tag="ps_sm")
                for c in range(DC):
                    nc.tensor.matmul(ps_s[:], lhs[:, c:c + 1],
                                     rhsT[:, c, h * 512:(h + 1) * 512],
                                     start=(c == 0), stop=(c == DC - 1))
                nc.vector.tensor_add(brow[:, h * 512:(h + 1) * 512], ps_s[:],
                                     logm[:, h * 512:(h + 1) * 512])
            bd = dram.tile([P * nt], F32)
            nc.sync.dma_start(bd[:].rearrange("(a n) -> a n", a=1), brow[:])
            nc.sync.dma_start(bcol[:], bd[:].rearrange("(t p) -> p t", p=P))

        # ---------- sim + exp (E_g: n-partition, bias b1) ----------
        E = big.tile([P, NT, M], mm_dtype)
        for t in range(NT):
            pe = psb.tile([P, M], F32, tag="ps_big")
            for h in range(M // 512):
                for c in range(DC):
                    nc.tensor.matmul(pe[:, h * 512:(h + 1) * 512],
                                     x1w3T[:, c, t * P:(t + 1) * P],
                                     x2T[:, c, h * 512:(h + 1) * 512],
                                     start=(c == 0), stop=(c == DC - 1))
            nc.scalar.activation(E[:, t, :], pe[:], EXP, bias=b1col[:, t:t + 1])

        # ---------- sim^T + exp (ET_g: m-partition, bias b2) ----------
        ET = big.tile([P, MT, N], mm_dtype)
        for t in range(MT):
            pe = psb.tile([P, N], F32, tag="ps_big")
            for h in range(N // 512):
                for c in range(DC):
                    nc.tensor.matmul(pe[:, h * 512:(h + 1) * 512],
                                     x2T[:, c, t * P:(t + 1) * P],
                                     x1w3T[:, c, h * 512:(h + 1) * 512],
                                     start=(c == 0), stop=(c == DC - 1))
            nc.scalar.activation(ET[:, t, :], pe[:], EXP, bias=b2col[:, t:t + 1])

        # ---------- denominators: den1[n] = sum_m ET_g, den2[m] = sum_n E_g ----------
        rden1 = const.tile([P, NT], F32)
        rden2 = const.tile([P, MT], F32)
        for (name, Esrc, rden, nt) in (("den1", ET, rden1, NT),
                                       ("den2", E, rden2, MT)):
            drow = rows.tile([1, P * nt], F32, tag=f"{name}row")
            for h in range(nt * P // 512):
                ps_d = pss.tile([1, 512], F32, tag="ps_sm")
                for k in range(NT):
                    nc.tensor.matmul(ps_d[:], ones_r[:],
                                     Esrc[:, k, h * 512:(h + 1) * 512],
                                     start=(k == 0), stop=(k == NT - 1))
                nc.vector.tensor_copy(drow[:, h * 512:(h + 1) * 512], ps_d[:])
            dd = dram.tile([P * nt], F32)
            nc.sync.dma_start(dd[:].rearrange("(a n) -> a n", a=1), drow[:])
            dcol = work.tile([P, nt], F32, tag="dcol")
            nc.sync.dma_start(dcol[:], dd[:].rearrange("(t p) -> p t", p=P))
            nc.vector.reciprocal(rden[:], dcol[:])

        # ---------- U_col -> q2c (plain, f32r) ----------
        Q2C = big.tile([P, MT, D], mm_dtype)
        for u in range(MT):
            pu = pss.tile([P, D], F32, tag="ps_sm")
            for k in range(NT):
                nc.tensor.matmul(pu[:], E[:, k, u * P:(u + 1) * P], x1aug[:, k, :],
                                 start=(k == 0), stop=(k == NT - 1))
            nc.vector.tensor_scalar_mul(Q2C[:, u, :], pu[:], rden2[:, u:u + 1])

        # ---------- U_row -> c2q ; out blocks 1, 2 ----------
        for t in range(NT):
            pr = pss.tile([P, D], F32, tag="ps_sm")
            for k in range(MT):
                nc.tensor.matmul(pr[:], ET[:, k, t * P:(t + 1) * P], x2aug[:, k, :],
                                 start=(k == 0), stop=(k == MT - 1))
            c2q = work.tile([P, D], F32, tag="ev")
            nc.vector.tensor_scalar_mul(c2q[:], pr[:], rden1[:, t:t + 1])
            nc.sync.dma_start(out_r[:, t, D:2 * D], c2q[:])
            prod = work.tile([P, D], F32, tag="ev")
            nc.vector.tensor_mul(prod[:], x1n[:, t, :], c2q[:])
            nc.sync.dma_start(out_r[:, t, 2 * D:3 * D], prod[:])

        # ---------- V -> q2c_att ; out block 3 ----------
        for t in range(NT):
            pv = pss.tile([P, D], F32, tag="ps_sm")
            for k in range(MT):
                nc.tensor.matmul(pv[:], ET[:, k, t * P:(t + 1) * P], Q2C[:, k, :],
                                 start=(k == 0), stop=(k == MT - 1))
            qa = work.tile([P, D], F32, tag="ev")
            nc.vector.tensor_scalar_mul(qa[:], pv[:], rden1[:, t:t + 1])
            prod = work.tile([P, D], F32, tag="ev")
            nc.vector.tensor_mul(prod[:], x1n[:, t, :], qa[:])
            nc.sync.dma_start(out_r[:, t, 3 * D:4 * D], prod[:])

    nc.compile()
    return nc


def _get_nc():
    if "nc" not in _CACHE:
        _CACHE["nc"] = _build()
    return _CACHE["nc"]


def _run(inputs, trace=False, trace_cores=None):
    nc = _get_nc()
    x1 = np.ascontiguousarray(np.asarray(inputs["x1"], dtype=np.float32))
    x2 = np.ascontiguousarray(np.asarray(inputs["x2"], dtype=np.float32))
    m1 = np.ascontiguousarray(np.asarray(inputs["x1_mask"]).astype(np.uint8))
    m2 = np.ascontiguousarray(np.asarray(inputs["x2_mask"]).astype(np.uint8))
    W = np.ascontiguousarray(np.asarray(inputs["W"], dtype=np.float32))
    in_maps = [
        {"x1": x1[i], "x2": x2[i], "x1_mask": m1[i], "x2_mask": m2[i], "W": W}
        for i in range(N_CORES)
    ]
    res = run_bass_kernel_spmd(nc, in_maps, core_ids=list(range(N_CORES)),
                               trace=trace, trace_cores=trace_cores)
    out = np.stack([res.results[i]["out"] for i in range(N_CORES)], axis=0)
    return out.astype(np.float32), res


def kernel(x1, x1_mask, x2, x2_mask, W, bias=None, **_kw):
    # bias is mathematically irrelevant: a global additive constant cancels in
    # both softmaxes, and every output term is softmax-weighted.
    out, _ = _run({"x1": x1, "x1_mask": x1_mask, "x2": x2, "x2_mask": x2_mask,
                   "W": W})
    return out
